# revision 30
# baseline (speedup 1.0000x reference)
"""Trainium2 Bass kernel for a dense transformer encoder layer.

Reference computation (per batch b):
    q = x.reshape(L, H, E)                       # H=16 heads, E=64
    scores = q @ q^T per head, scaled softmax    # A = softmax(s/8)
    new_x  = concat_h(A_h @ q_h)                 # [L, D]
    x1 = LN(x + new_x; g1, be1)
    y  = relu(x1 @ w1^T + b1) @ w2^T + b2
    out = LN(x1 + y; g2, be2)

Sharding: pure data parallel over (batch, seq-half): core c handles
batch c//2, query rows [(c%2)*1024, +1024).  Keys span the full sequence
of that batch, so every core gets the whole x[b] (queries reordered
first) and the full FFN weights.  No device collectives.

Per-core schedule:
  - All big attention operands are pre-laid-out on the HOST: x^T tiles
    ([d, s] bf16 for the scores matmuls), the fp8 [V|1] array (value
    rows interleaved with a ones column per head, for the AV matmuls),
    and an fp32 copy of the query rows that seeds the attention
    residual accumulator.  This removes all on-device layout work
    (transposes + copies) from the attention phase.
  - Heads are processed in PAIRS: head 2t lives in PE rows 0-63 and
    head 2t+1 in rows 64-127 of the same x^T d-tile, so their scores
    matmuls (K=64 contraction) run CONCURRENTLY in separate PE
    row-groups (tile_position row tiling).
  - exp(s/8 - 1) is written directly in fp8-e4m3: ACT exp for most
    tiles, and a Schraudolph fast-exp on the DVE (affine int ->
    uint8 bitcast IS e4m3 exp, with negative results clamped to +0 by
    the uint8 saturation) for the rest -- the split keeps both engines
    busy since exp throughput paces the whole attention phase.
  - AV matmuls run in fp8 DoubleRow mode: 256-key contraction per
    instruction (the [V|1] array pairs adjacent key tiles in its free
    dim), halving PE time.  The ones column emits the softmax
    denominator in row 64 of U^T = [V|1]^T E^T.
  - U^T tiles are PE-transposed back to [l, 65] (bf16); one batched
    reciprocal per head and one fused scalar_tensor_tensor per
    (head, l-tile) divide by the denominator and add the fp32 x
    residual in place.
  - LayerNorm = bn_stats/bn_aggr + sqrt + one ACT Identity pass.
  - FFN1 in bf16 (streamed weights, prefetched during attention);
    FFN2 in fp8 DoubleRow with weights scaled x32 on the host (the
    1/32 folds into the bias pass).
"""

import numpy as np

import concourse.bass as bass
import concourse.tile as tile
from concourse import bacc
from concourse import mybir
from concourse.masks import make_identity

F32 = mybir.dt.float32
BF16 = mybir.dt.bfloat16
FP8 = mybir.dt.float8e4
U8 = mybir.dt.uint8
EXP = mybir.ActivationFunctionType.Exp
RELU = mybir.ActivationFunctionType.Relu
SQRT = mybir.ActivationFunctionType.Sqrt
IDENT = mybir.ActivationFunctionType.Identity
ADD = mybir.AluOpType.add
MUL = mybir.AluOpType.mult
DR = mybir.MatmulPerfMode.DoubleRow

LN_EPS = 1e-5
ESHIFT = 3.5    # logit shift: et = exp(s/8 - ESHIFT); keeps fp8 et < 240
LOG2E = float(np.log2(np.e))
# Schraudolph fast-exp in e4m3 bits: et = bitcast_fp8(uint8(A*s + B));
# uint8 saturation clamps negative results (tiny et) to +0.  Only used on
# cross-half key tiles: no diagonal, so the affine result stays in [0, 110].
FEXP_A8 = LOG2E                      # d(bits)/ds = 8 * log2e / 8
FEXP_B8 = float(8 * 7 - 0.5 + 0.344 - 8 * LOG2E * ESHIFT)
# Schraudolph in bf16 bits via int16: d(bits)/ds = 2^7 * log2e / 8
FEXP_A16 = float(2 ** 7 * LOG2E / 8.0)
FEXP_B16 = float(127 * 2 ** 7 - 5.6 - 2 ** 7 * LOG2E * ESHIFT)
I16 = mybir.dt.int16
E = 64          # head dim
W = E + 1       # head dim + ones column
P = 128         # partitions
MDVE32 = 11     # of the 32 cross-half exp slabs per pair, DVE takes this many
FFN2_FP8 = True
NFP8_FT2 = 8    # f-tile pairs of the FFN2 contraction done in fp8 DoubleRow
S2 = 32.0       # host-side scale on w2 in fp8 (folded out in the bias pass)


def build_program(S=2048, D=1024, F=4096, affine1=False, affine2=False):
    """Build the per-core Bass program.  S = full seq len; queries are the
    first Lq = S//2 rows of the reordered sequence."""
    H = D // E
    NP = H // 2          # head pairs
    Lq = S // 2
    ST = S // P          # key tiles
    NU2 = ST // 2        # key tile pairs (DoubleRow contracts 2 at a time)
    LT = Lq // P         # query row tiles
    DT = D // P          # d chunks
    FT = F // P          # f tiles
    SL = 512             # matmul moving slab (one PSUM bank of fp32)
    NSL = Lq // SL
    GS = min(512, D)     # bn_stats subgroup size
    W1_PREF = 6          # w1 f-tiles prefetched during attention

    nc = bacc.Bacc("TRN2")

    xT_d = nc.dram_tensor("xT", [DT, P, S], BF16, kind="ExternalInput")
    v8_d = nc.dram_tensor("v8", [ST // 2, P, H * W], FP8,
                          kind="ExternalInput")
    v16_d = nc.dram_tensor("v16", [ST // 2, P, H * W], BF16,
                           kind="ExternalInput")
    xq32_d = nc.dram_tensor("xq32", [LT, P, D], F32, kind="ExternalInput")
    w1t = nc.dram_tensor("w1t", [FT, P, D], BF16, kind="ExternalInput")
    NF8 = 2 * NFP8_FT2 if FFN2_FP8 else 0   # f-tiles of FFN2 in fp8
    w2t8 = w2t16 = None
    if NF8:
        w2t8 = nc.dram_tensor("w2t8", [DT, P, NF8 * P], FP8,
                              kind="ExternalInput")
    if NF8 < FT:
        w2t16 = nc.dram_tensor("w2t16", [DT, P, (FT - NF8) * P], BF16,
                               kind="ExternalInput")
    b1 = nc.dram_tensor("b1", [F], F32, kind="ExternalInput")
    b2 = nc.dram_tensor("b2", [D], F32, kind="ExternalInput")
    g1 = nc.dram_tensor("g1", [D], F32, kind="ExternalInput")
    be1 = nc.dram_tensor("be1", [D], F32, kind="ExternalInput")
    g2 = nc.dram_tensor("g2", [D], F32, kind="ExternalInput")
    be2 = nc.dram_tensor("be2", [D], F32, kind="ExternalInput")
    out = nc.dram_tensor("out", [Lq, D], F32, kind="ExternalOutput")
    import os
    dbg = None
    if os.environ.get("K_DBG"):
        dbg = nc.dram_tensor("dbg", [LT, P, D], F32, kind="ExternalOutput")

    def bcast(dram_vec):
        a = dram_vec[:]
        return bass.AP(tensor=a.tensor, offset=a.offset, ap=[[0, P]] + a.ap)

    with tile.TileContext(nc) as tc:
        with (
            tc.tile_pool(name="persist", bufs=1) as persist,
            tc.tile_pool(name="small", bufs=8) as small,
            tc.tile_pool(name="w1p", bufs=W1_PREF) as w1p,
        ):
            ident16 = persist.tile([P, P], BF16)
            make_identity(nc, ident16)
            b1s = persist.tile([P, FT], F32)
            nc.sync.dma_start(out=b1s, in_=b1[:].rearrange("(t p) -> p t", p=P))
            b2s = persist.tile([P, DT], F32)
            nc.sync.dma_start(out=b2s, in_=b2[:].rearrange("(t p) -> p t", p=P))
            epst = persist.tile([P, 1], F32)
            nc.vector.memset(epst, LN_EPS)
            esht = persist.tile([P, 1], F32)
            nc.vector.memset(esht, -ESHIFT)
            zer65 = persist.tile([P, W], BF16)
            nc.vector.memset(zer65, 0.0)
            # new_x starts as the fp32 query rows (the attention residual),
            # accumulates the attention output, and after LN1 is reused as
            # the residual-2 accumulator.
            new_x = persist.tile([P, LT, D], F32)
            for lt in range(LT):
                nc.sync.dma_start(out=new_x[:, lt, :], in_=xq32_d[lt])

            # ---------------- attention ----------------
            with (
                tc.tile_pool(name="attn_sb", bufs=1) as asb,
                tc.tile_pool(name="etp", bufs=1) as etp,
                tc.tile_pool(name="utsp", bufs=4) as utsp,
            ):
                xTs = asb.tile([P, DT, S], BF16)
                for t in range(DT):
                    nc.sync.dma_start(out=xTs[:, t, :], in_=xT_d[t])
                v16s = asb.tile([P, ST // 2, H, W], BF16)
                for u in range(ST // 2):
                    nc.sync.dma_start(
                        out=v16s[:, u, :, :],
                        in_=v16_d[u].rearrange("p (h w) -> p h w", w=W))
                v8s = asb.tile([P, ST // 2, H, W], FP8)
                for u in range(ST // 2):
                    nc.sync.dma_start(
                        out=v8s[:, u, :, :],
                        in_=v8_d[u].rearrange("p (h w) -> p h w", w=W))

                # prefetch the first w1 f-tiles so FFN1 starts instantly
                w1tiles = []
                for ft in range(W1_PREF):
                    wt = w1p.tile([P, DT, P], BF16, tag="w1t")
                    nc.sync.dma_start(out=wt, in_=w1t[ft].rearrange(
                        "p (t m) -> p t m", m=P))
                    w1tiles.append(wt)

                with (
                    tc.tile_pool(name="scp", bufs=1, space="PSUM") as scp,
                    tc.tile_pool(name="utp", bufs=2, space="PSUM") as utp,
                    tc.tile_pool(name="upp", bufs=2, space="PSUM") as upp,
                ):
                    # PE warmup: ~10us of back-to-back matmuls (gated on the
                    # first xT DMA) flips the HAM clock gate to 8/8 just as
                    # the first scores issue
                    wsc = scp.tile([P, SL], F32, tag="sc1", bufs=2)
                    for _ in range(46):
                        nc.tensor.matmul(
                            wsc, xTs[:, 0, 0:P], xTs[:, 0, 0:SL],
                            start=True, stop=True)

                    ets_cur = {}    # (h01, u2) -> et tile consumed this slot
                    ek = [0]        # cross-half exp slab round-robin counter

                    def emit_exp(sc, dst, fp8, has_diag):
                        if has_diag:
                            # diagonal weights dominate the softmax; keep
                            # their exp at full ACT accuracy
                            nc.scalar.activation(
                                out=dst, in_=sc, func=EXP,
                                scale=1.0 / 8.0, bias=esht)
                        elif not fp8:
                            nc.vector.tensor_scalar(
                                out=dst.bitcast(I16), in0=sc,
                                scalar1=FEXP_A16, scalar2=FEXP_B16,
                                op0=MUL, op1=ADD)
                        else:
                            k = ek[0] % 32
                            ek[0] += 1
                            if k % 3 == 1:
                                nc.vector.tensor_scalar(
                                    out=dst.bitcast(U8), in0=sc,
                                    scalar1=FEXP_A8, scalar2=FEXP_B8,
                                    op0=MUL, op1=ADD)
                            else:
                                nc.scalar.activation(
                                    out=dst, in_=sc, func=EXP,
                                    scale=1.0 / 8.0, bias=esht)

                    epi_prev = None   # (pair, uts dict) awaiting epilogue
                    for slot in range(-1, NP + 1):
                        pN = slot + 1   # pair whose scores/exp run this slot
                        pA = slot       # pair whose AV (both slabs) runs now
                        ets_new = {}
                        uts_all = {}

                        def av_step(u2, s, uls):
                            for h01 in range(2):
                                if u2 < NU2 // 2:
                                    for j in range(2):
                                        nc.tensor.matmul(
                                            uls[h01],
                                            v16s[:, 2 * u2 + j,
                                                 2 * pA + h01, :],
                                            ets_cur[(h01, u2)][
                                                :, j, s * SL:(s + 1) * SL],
                                            start=(u2 == 0 and j == 0),
                                            stop=False)
                                    continue
                                nc.tensor.matmul(
                                    uls[h01],
                                    v8s[:, 2 * (u2 - NU2 // 2):
                                        2 * (u2 - NU2 // 2) + 2,
                                        2 * pA + h01, :],
                                    ets_cur[(h01, u2)][:, :,
                                                       s * SL:(s + 1) * SL],
                                    start=False,
                                    stop=(u2 == NU2 - 1),
                                    perf_mode=DR)

                        # epilogue of the PREVIOUS pair, broken into small
                        # ops and spread across this slot so the DVE FIFO
                        # never blocks the exp pipeline for long
                        epi_ops = []
                        if epi_prev is not None:
                            pE, utsE = epi_prev
                            for h01 in range(2):
                                g = 2 * pE + h01
                                up = upp.tile([P, LT, W + 3], BF16,
                                              tag="up", bufs=1,
                                              name=f"up{slot}{h01}")
                                rz8 = small.tile([P, LT], F32, tag="rz8",
                                                 name=f"rz{slot}{h01}")

                                def mk_t(h01, lt, up=up):
                                    def go():
                                        src_ = utsE[(h01, lt // (LT // 2))]
                                        lo = (lt % (LT // 2)) * P
                                        nc.tensor.transpose(
                                            up[:, lt, 0:W],
                                            src_[:, lo:lo + P],
                                            ident16[0:W, 0:W])
                                    return go

                                def mk_r(up=up, rz8=rz8):
                                    def go():
                                        nc.vector.reciprocal(
                                            out=rz8, in_=up[:, :, E])
                                    return go

                                def mk_s(lt, g=g, up=up, rz8=rz8):
                                    def go():
                                        nc.vector.scalar_tensor_tensor(
                                            out=new_x[:, lt,
                                                      g * E:(g + 1) * E],
                                            in0=up[:, lt, 0:E],
                                            scalar=rz8[:, lt:lt + 1],
                                            in1=new_x[:, lt,
                                                      g * E:(g + 1) * E],
                                            op0=MUL, op1=ADD)
                                    return go
                                for lt in range(LT):
                                    epi_ops.append(mk_t(h01, lt))
                                epi_ops.append(mk_r())
                                for lt in range(LT):
                                    epi_ops.append(mk_s(lt))

                        if 0 <= pA < NP:
                            uls = [utp.tile([W, SL], F32, tag="ut",
                                            name=f"ut{slot}a{i}")
                                   for i in range(2)]
                        for u2 in range(NU2):
                            if 0 <= pA < NP and u2 == NU2 // 2:
                                # slab 0 done: drain it and switch the
                                # accumulators to slab 1
                                for h01 in range(2):
                                    uts = utsp.tile([W, SL], BF16,
                                                    tag="uts", bufs=8)
                                    nc.vector.tensor_copy(out=uts,
                                                          in_=uls[h01])
                                    uts_all[(h01, 0)] = uts
                                uls = [utp.tile([W, SL], F32, tag="ut",
                                                name=f"ut{slot}b{i}")
                                       for i in range(2)]
                            if pN < NP:
                                for j in range(2):
                                    u = 2 * u2 + j
                                    fp8 = u2 >= NU2 // 2
                                    for h01 in range(2):
                                        ro = h01 * E
                                        if j == 0:
                                            et = etp.tile(
                                                [P, 2, Lq],
                                                FP8 if fp8 else BF16,
                                                tag="et8" if fp8 else "et16",
                                                bufs=2 * ST // 2,
                                                name=f"et{slot}_{u2}_{h01}")
                                            ets_new[(h01, u2)] = et
                                        et = ets_new[(h01, u2)]
                                        for s in range(NSL):
                                            sc = scp.tile([P, SL], F32,
                                                          tag=f"sc{h01}",
                                                          bufs=3 - h01,
                                                          name=f"sc{h01}_{s}")
                                            nc.tensor.matmul(
                                                sc,
                                                xTs[ro:ro + E, pN,
                                                    u * P:(u + 1) * P],
                                                xTs[ro:ro + E, pN,
                                                    s * SL:(s + 1) * SL],
                                                start=True, stop=True)
                                            emit_exp(
                                                sc,
                                                et[:, j,
                                                   s * SL:(s + 1) * SL],
                                                fp8,
                                                (not fp8) and u // (NU2 // 2)
                                                == s)
                                    if 0 <= pA < NP and j == 0:
                                        # two AV steps of the previous pair
                                        # per iteration: slab 0 in the first
                                        # half of the slot, slab 1 after
                                        s_, base = ((0, 0) if u2 < NU2 // 2
                                                    else (1, NU2))
                                        av_step(2 * u2 - base, s_, uls)
                                        av_step(2 * u2 + 1 - base, s_, uls)
                            elif 0 <= pA < NP:
                                s_, base = ((0, 0) if u2 < NU2 // 2
                                            else (1, NU2))
                                av_step(2 * u2 - base, s_, uls)
                                av_step(2 * u2 + 1 - base, s_, uls)
                            # spread epilogue pops over the PE-sparse
                            # cross-half iterations
                            if u2 >= NU2 // 2 - 1:
                                for _ in range(7):
                                    if epi_ops:
                                        epi_ops.pop(0)()
                        while epi_ops:
                            epi_ops.pop(0)()
                        if 0 <= pA < NP:
                            for h01 in range(2):
                                uts = utsp.tile([W, SL], BF16, tag="uts",
                                                bufs=8)
                                nc.vector.tensor_copy(out=uts, in_=uls[h01])
                                uts_all[(h01, 1)] = uts
                            epi_prev = (pA, uts_all)
                        # the ets produced this slot become next slot's input
                        if pN < NP:
                            ets_cur = ets_new

            if dbg is not None:
                for lt in range(LT):
                    nc.sync.dma_start(out=dbg[lt], in_=new_x[:, lt, :])
            # ---------------- LN1 + FFN ----------------
            with (
                tc.tile_pool(name="ffn_sb", bufs=1) as fsb,
                tc.tile_pool(name="w2p", bufs=3) as w2p,
                tc.tile_pool(name="ysbp", bufs=2) as ysbp,
                tc.tile_pool(name="outp", bufs=2) as outp,
            ):
                # residual-1 complete in new_x; LN1 -> x1b (bf16).
                x1b = fsb.tile([P, LT, D], BF16)
                gb1 = beb1 = None
                if affine1:
                    gb1 = fsb.tile([P, D], F32)
                    nc.gpsimd.dma_start(out=gb1, in_=bcast(g1))
                    beb1 = fsb.tile([P, D], F32)
                    nc.gpsimd.dma_start(out=beb1, in_=bcast(be1))
                x1T = fsb.tile([P, DT, Lq], BF16)
                x1tp_ctx = tc.tile_pool(name="x1tp", bufs=4, space="PSUM")
                x1tp = x1tp_ctx.__enter__()

                def emit_ln1(lt):
                    _layer_norm(nc, small, x1b[:, lt, :], new_x[:, lt, :],
                                gb1, beb1, epst, GS, affine1,
                                on_dve=(lt % 2 == 1))

                def emit_x1t(lt, c):
                    tp = x1tp.tile([P, P], BF16, tag="tp",
                                   name=f"tp{lt}{c}")
                    nc.tensor.transpose(
                        tp, x1b[:, lt, c * P:(c + 1) * P], ident16)
                    nc.vector.tensor_copy(
                        out=x1T[:, c, lt * P:(lt + 1) * P], in_=tp)

                rest = []
                for lt in range(LT):
                    if lt < LT // 2:
                        emit_ln1(lt)
                    else:
                        rest.append((emit_ln1, lt))
                for lt in range(LT // 2):
                    for c in range(DT):
                        emit_x1t(lt, c)
                for lt in range(LT // 2, LT):
                    for c in range(DT):
                        rest.append((emit_x1t, lt, c))

                hts = hts16 = None
                if NF8:
                    hts = fsb.tile([P, NF8, Lq], FP8)
                if NF8 < FT:
                    hts16 = fsb.tile([P, FT - NF8, Lq], BF16)
                with tc.tile_pool(name="hpp", bufs=4, space="PSUM") as hpp:
                    for s in range(NSL):
                        for ft in range(FT):
                            # second-half LN1 + transposes ride along pass 0
                            for _ in range(2):
                                if rest:
                                    op = rest.pop(0)
                                    op[0](*op[1:])
                            if s == 0 and ft < W1_PREF:
                                wt = w1tiles[ft]
                            else:
                                wt = w1p.tile([P, DT, P], BF16, tag="w1t")
                                nc.sync.dma_start(
                                    out=wt, in_=w1t[ft].rearrange(
                                        "p (t m) -> p t m", m=P))
                            hp = hpp.tile([P, SL], F32, tag="hp")
                            for dc in range(DT):
                                nc.tensor.matmul(
                                    hp, wt[:, dc, :],
                                    x1T[:, dc, s * SL:(s + 1) * SL],
                                    start=(dc == 0), stop=(dc == DT - 1))
                            hdst = (hts[:, ft, :] if ft < NF8
                                    else hts16[:, ft - NF8, :])
                            nc.scalar.activation(
                                out=hdst[:, s * SL:(s + 1) * SL],
                                in_=hp, func=RELU,
                                bias=b1s[:, ft:ft + 1])
                x1tp_ctx.__exit__(None, None, None)
                gb2 = beb2 = None
                if affine2:
                    gb2 = fsb.tile([P, D], F32)
                    nc.gpsimd.dma_start(out=gb2, in_=bcast(g2))
                    beb2 = fsb.tile([P, D], F32)
                    nc.gpsimd.dma_start(out=beb2, in_=bcast(be2))

                with (
                    tc.tile_pool(name="ypp", bufs=4, space="PSUM") as ypp,
                    tc.tile_pool(name="tpp", bufs=4, space="PSUM") as tpp,
                ):
                    for s in range(NSL):
                        for dt in range(DT):
                            w2s = w2s16 = None
                            if NF8:
                                w2s = w2p.tile([P, NF8 // 2, 2, P], FP8,
                                               tag="w2t")
                                nc.sync.dma_start(
                                    out=w2s, in_=w2t8[dt].rearrange(
                                        "p (a b m) -> p a b m", b=2, m=P))
                            if NF8 < FT:
                                w2s16 = w2p.tile([P, FT - NF8, P], BF16,
                                                 tag="w2t16")
                                nc.sync.dma_start(
                                    out=w2s16, in_=w2t16[dt].rearrange(
                                        "p (t m) -> p t m", m=P))
                            yp = ypp.tile([P, SL], F32, tag="yp")
                            for ft2 in range(NF8 // 2):
                                nc.tensor.matmul(
                                    yp, w2s[:, ft2, :, :],
                                    hts[:, 2 * ft2:2 * ft2 + 2,
                                        s * SL:(s + 1) * SL],
                                    start=(ft2 == 0),
                                    stop=(NF8 == FT and
                                          ft2 == NF8 // 2 - 1),
                                    perf_mode=DR)
                            for ft in range(NF8, FT):
                                nc.tensor.matmul(
                                    yp, w2s16[:, ft - NF8, :],
                                    hts16[:, ft - NF8,
                                          s * SL:(s + 1) * SL],
                                    start=(ft == 0), stop=(ft == FT - 1))
                            ysb = ysbp.tile([P, SL], BF16, tag="ysb")
                            nc.vector.tensor_scalar(
                                out=ysb, in0=yp,
                                scalar1=1.0 / S2,
                                scalar2=b2s[:, dt:dt + 1],
                                op0=MUL, op1=ADD)
                            # transpose y back to [l, d], add the x1 residual
                            for lq in range(SL // P):
                                lt = s * (SL // P) + lq
                                tp = tpp.tile([P, P], BF16)
                                nc.tensor.transpose(
                                    tp, ysb[:, lq * P:(lq + 1) * P], ident16)
                                nc.vector.scalar_tensor_tensor(
                                    out=new_x[:, lt, dt * P:(dt + 1) * P],
                                    in0=tp, scalar=1.0,
                                    in1=x1b[:, lt, dt * P:(dt + 1) * P],
                                    op0=MUL, op1=ADD)

                        # this token half is complete: LN2 + store while the
                        # other half's FFN2 runs
                        for lq in range(SL // P):
                            lt = s * (SL // P) + lq
                            ot = outp.tile([P, D], F32, tag="ot")
                            _layer_norm(nc, small, ot, new_x[:, lt, :],
                                        gb2, beb2, epst, GS, affine2,
                                        on_dve=(lt % 2 == 1))
                            nc.sync.dma_start(
                                out=out[lt * P:(lt + 1) * P, 0:D // 2],
                                in_=ot[:, 0:D // 2])
                            nc.sync.dma_start(
                                out=out[lt * P:(lt + 1) * P, D // 2:D],
                                in_=ot[:, D // 2:D])

    nc.finalize()
    return nc


def _layer_norm(nc, small, out_ap, x_ap, gb, beb, epst, GS, affine,
                on_dve=False):
    """out = (x - mean(x)) * rsqrt(var(x) + eps) [* g + be] over free dim.
    The normalize pass runs on ACT by default, or DVE (on_dve) so
    consecutive LNs can alternate engines."""
    D = x_ap.shape[-1]
    ngr = D // GS
    st = small.tile([P, ngr, 6], F32, tag="bnst")
    xg = x_ap.rearrange("p (g k) -> p g k", k=GS)
    for g in range(ngr):
        nc.vector.bn_stats(out=st[:, g, :], in_=xg[:, g, :])
    mv = small.tile([P, 2], F32, tag="bnmv")
    nc.vector.bn_aggr(out=mv, in_=st)
    sd = small.tile([P, 1], F32, tag="sd")
    nc.scalar.activation(out=sd, in_=mv[:, 1:2], func=SQRT, bias=epst)
    rstd = small.tile([P, 1], F32, tag="rstd")
    nc.vector.reciprocal(out=rstd, in_=sd)
    dst = out_ap
    if affine:
        dst = small.tile([P, D], F32, tag="xn", bufs=2)
    if on_dve:
        nc.vector.tensor_scalar(
            out=dst, in0=x_ap, scalar1=mv[:, 0:1], scalar2=rstd,
            op0=mybir.AluOpType.subtract, op1=MUL)
    else:
        nmr = small.tile([P, 1], F32, tag="nmr")
        nc.vector.scalar_tensor_tensor(
            out=nmr, in0=mv[:, 0:1], scalar=-1.0, in1=rstd,
            op0=MUL, op1=MUL)
        nc.scalar.activation(out=dst, in_=x_ap, func=IDENT,
                             bias=nmr, scale=rstd)
    if affine:
        nc.vector.tensor_mul(out=dst, in0=dst, in1=gb)
        nc.vector.tensor_add(out=out_ap, in0=dst, in1=beb)


# ---------------------------------------------------------------------------
# host side
# ---------------------------------------------------------------------------

_PROG_CACHE = {}


def get_program(S=2048, D=1024, F=4096, affine1=False, affine2=False):
    key = (S, D, F, affine1, affine2)
    if key not in _PROG_CACHE:
        _PROG_CACHE[key] = build_program(S, D, F, affine1, affine2)
    return _PROG_CACHE[key]


def make_in_maps(x, w1, b1, w2, b2, g1, be1, g2, be2, n_cores=8):
    import ml_dtypes
    FP8NP = ml_dtypes.float8_e4m3fn
    B, L, D = x.shape
    F = w1.shape[0]
    S = L
    H = D // E
    Lq = L // 2
    ST, LT, DT, FT = S // P, Lq // P, D // P, F // P
    x = np.asarray(x, np.float32)
    # w1t[ft, p, dc*128+m] = w1[ft*128+m, dc*128+p]
    w1t = np.ascontiguousarray(
        w1.astype(np.float32).reshape(FT, P, DT, P).transpose(0, 3, 2, 1)
        .reshape(FT, P, D)).astype(ml_dtypes.bfloat16)
    # w2 transposed [dt, p, ft*128+m] = w2[dt*128+m, ft*128+p] * S2,
    # first NF8 f-tiles in fp8 (DoubleRow), the rest in bf16 (the *32
    # scale is exact in bf16 and lets one uniform unscale serve both)
    NF8 = 2 * NFP8_FT2 if FFN2_FP8 else 0
    w2m = (w2.astype(np.float32).reshape(DT, P, FT, P)
           .transpose(0, 3, 2, 1).reshape(DT, P, FT, P)) * S2
    common = dict(w1t=w1t, b1=b1, b2=b2, g1=g1, be1=be1, g2=g2, be2=be2)
    if NF8:
        common["w2t8"] = np.ascontiguousarray(
            w2m[:, :, :NF8]).astype(FP8NP).reshape(DT, P, NF8 * P)
    if NF8 < FT:
        common["w2t16"] = np.ascontiguousarray(
            w2m[:, :, NF8:]).astype(ml_dtypes.bfloat16).reshape(
                DT, P, (FT - NF8) * P)
    in_maps = []
    for c in range(n_cores):
        b, half = c // 2, c % 2
        lo = half * Lq
        xq = x[b, lo:lo + Lq]
        xo = x[b, Lq - lo:2 * Lq - lo]
        xbl = np.concatenate([xq, xo], axis=0)
        # xT[t, p, s] = xbl[s, t*128+p]
        xT = np.ascontiguousarray(
            xbl.reshape(S, DT, P).transpose(1, 2, 0)).astype(
                ml_dtypes.bfloat16)
        # v[u, p, h, :] = [xbl[u*128+p, h*64:(h+1)*64] | 1.0]
        # same-half key tiles (u < ST/2, contain the diagonal) in bf16,
        # cross-half tiles in fp8.
        va = np.ones((ST, P, H, W), np.float32)
        va[:, :, :, 0:E] = xbl.reshape(ST, P, H, E)
        v16 = np.ascontiguousarray(va[:ST // 2]).astype(
            ml_dtypes.bfloat16).reshape(ST // 2, P, H * W)
        v8 = np.ascontiguousarray(va[ST // 2:]).astype(FP8NP).reshape(
            ST // 2, P, H * W)
        xq32 = np.ascontiguousarray(xbl[0:Lq].reshape(LT, P, D))
        in_maps.append(dict(xT=xT, v16=v16, v8=v8, xq32=xq32, **common))
    return in_maps


def kernel(x, w1, b1, w2, b2, g1, be1, g2, be2):
    from concourse.bass_utils import run_bass_kernel_spmd

    x = np.asarray(x, dtype=np.float32)
    B, L, D = x.shape
    F = w1.shape[0]
    Lq = L // 2
    n_cores = 2 * B
    g1 = np.asarray(g1, np.float32)
    be1 = np.asarray(be1, np.float32)
    g2 = np.asarray(g2, np.float32)
    be2 = np.asarray(be2, np.float32)
    affine1 = not (np.all(g1 == 1.0) and np.all(be1 == 0.0))
    affine2 = not (np.all(g2 == 1.0) and np.all(be2 == 0.0))
    nc = get_program(L, D, F, affine1, affine2)
    in_maps = make_in_maps(x, np.asarray(w1, np.float32),
                           np.asarray(b1, np.float32),
                           np.asarray(w2, np.float32),
                           np.asarray(b2, np.float32),
                           g1, be1, g2, be2, n_cores)
    res = run_bass_kernel_spmd(nc, in_maps, core_ids=list(range(n_cores)))
    outp = np.empty((B, L, D), dtype=np.float32)
    for c in range(n_cores):
        b, half = c // 2, c % 2
        outp[b, half * Lq:(half + 1) * Lq] = res.results[c]["out"]
    return outp


# revision 31
# speedup vs baseline: 1.1614x; 1.1614x over previous
"""Trainium2 Bass kernel for a dense transformer encoder layer.

Reference computation (per batch b):
    q = x.reshape(L, H, E)                       # H=16 heads, E=64
    scores = q @ q^T per head, scaled softmax    # A = softmax(s/8)
    new_x  = concat_h(A_h @ q_h)                 # [L, D]
    x1 = LN(x + new_x; g1, be1)
    y  = relu(x1 @ w1^T + b1) @ w2^T + b2
    out = LN(x1 + y; g2, be2)

Sharding: pure data parallel over (batch, seq-half): core c handles
batch c//2, query rows [(c%2)*1024, +1024).  Keys span the full sequence
of that batch, so every core gets the whole x[b] (queries reordered
first) and the full FFN weights.  No device collectives.

Per-core schedule:
  - All big attention operands are pre-laid-out on the HOST: x^T tiles
    ([d, s] bf16 for the scores matmuls), the fp8 [V|1] array (value
    rows interleaved with a ones column per head, for the AV matmuls),
    and an fp32 copy of the query rows that seeds the attention
    residual accumulator.  This removes all on-device layout work
    (transposes + copies) from the attention phase.
  - Heads are processed in PAIRS: head 2t lives in PE rows 0-63 and
    head 2t+1 in rows 64-127 of the same x^T d-tile, so their scores
    matmuls (K=64 contraction) run CONCURRENTLY in separate PE
    row-groups (tile_position row tiling).
  - exp(s/8 - 1) is written directly in fp8-e4m3: ACT exp for most
    tiles, and a Schraudolph fast-exp on the DVE (affine int ->
    uint8 bitcast IS e4m3 exp, with negative results clamped to +0 by
    the uint8 saturation) for the rest -- the split keeps both engines
    busy since exp throughput paces the whole attention phase.
  - AV matmuls run in fp8 DoubleRow mode: 256-key contraction per
    instruction (the [V|1] array pairs adjacent key tiles in its free
    dim), halving PE time.  The ones column emits the softmax
    denominator in row 64 of U^T = [V|1]^T E^T.
  - U^T tiles are PE-transposed back to [l, 65] (bf16); one batched
    reciprocal per head and one fused scalar_tensor_tensor per
    (head, l-tile) divide by the denominator and add the fp32 x
    residual in place.
  - LayerNorm = bn_stats/bn_aggr + sqrt + one ACT Identity pass.
  - FFN1 in bf16 (streamed weights, prefetched during attention);
    FFN2 in fp8 DoubleRow with weights scaled x32 on the host (the
    1/32 folds into the bias pass).
"""

import numpy as np

import concourse.bass as bass
import concourse.tile as tile
from concourse import bacc
from concourse import mybir
from concourse.masks import make_identity

F32 = mybir.dt.float32
BF16 = mybir.dt.bfloat16
FP8 = mybir.dt.float8e4
U8 = mybir.dt.uint8
EXP = mybir.ActivationFunctionType.Exp
RELU = mybir.ActivationFunctionType.Relu
SQRT = mybir.ActivationFunctionType.Sqrt
IDENT = mybir.ActivationFunctionType.Identity
ADD = mybir.AluOpType.add
MUL = mybir.AluOpType.mult
DR = mybir.MatmulPerfMode.DoubleRow

LN_EPS = 1e-5
ESHIFT = 3.5    # logit shift: et = exp(s/8 - ESHIFT); keeps fp8 et < 240
LOG2E = float(np.log2(np.e))
# Schraudolph fast-exp in e4m3 bits: et = bitcast_fp8(uint8(A*s + B));
# uint8 saturation clamps negative results (tiny et) to +0.  Only used on
# cross-half key tiles: no diagonal, so the affine result stays in [0, 110].
FEXP_A8 = LOG2E                      # d(bits)/ds = 8 * log2e / 8
FEXP_B8 = float(8 * 7 - 0.5 + 0.344 - 8 * LOG2E * ESHIFT)
# Schraudolph in bf16 bits via int16: d(bits)/ds = 2^7 * log2e / 8
FEXP_A16 = float(2 ** 7 * LOG2E / 8.0)
FEXP_B16 = float(127 * 2 ** 7 - 5.6 - 2 ** 7 * LOG2E * ESHIFT)
I16 = mybir.dt.int16
E = 64          # head dim
W = E + 1       # head dim + ones column
P = 128         # partitions
MDVE32 = 11     # of the 32 cross-half exp slabs per pair, DVE takes this many
FFN2_FP8 = True
NFP8_FT2 = 8    # f-tile pairs of the FFN2 contraction done in fp8 DoubleRow
S2 = 32.0       # host-side scale on w2 in fp8 (folded out in the bias pass)


def build_program(S=2048, D=1024, F=4096, affine1=False, affine2=False):
    """Build the per-core Bass program.  S = full seq len; queries are the
    first Lq = S//2 rows of the reordered sequence."""
    H = D // E
    NP = H // 2          # head pairs
    Lq = S // 2
    ST = S // P          # key tiles
    NU2 = ST // 2        # key tile pairs (DoubleRow contracts 2 at a time)
    LT = Lq // P         # query row tiles
    DT = D // P          # d chunks
    FT = F // P          # f tiles
    SL = 512             # matmul moving slab (one PSUM bank of fp32)
    NSL = Lq // SL
    GS = min(512, D)     # bn_stats subgroup size
    W1_PREF = 6          # w1 f-tiles prefetched during attention

    nc = bacc.Bacc("TRN2")

    xT_d = nc.dram_tensor("xT", [DT, P, S], BF16, kind="ExternalInput")
    v8_d = nc.dram_tensor("v8", [ST // 2, P, H * W], FP8,
                          kind="ExternalInput")
    v16_d = nc.dram_tensor("v16", [ST // 2, P, H * W], BF16,
                           kind="ExternalInput")
    xq32_d = nc.dram_tensor("xq32", [LT, P, D], F32, kind="ExternalInput")
    w1t = nc.dram_tensor("w1t", [FT, P, D], BF16, kind="ExternalInput")
    NF8 = 2 * NFP8_FT2 if FFN2_FP8 else 0   # f-tiles of FFN2 in fp8
    w2t8 = w2t16 = None
    if NF8:
        w2t8 = nc.dram_tensor("w2t8", [DT, P, NF8 * P], FP8,
                              kind="ExternalInput")
    if NF8 < FT:
        w2t16 = nc.dram_tensor("w2t16", [DT, P, (FT - NF8) * P], BF16,
                               kind="ExternalInput")
    b1 = nc.dram_tensor("b1", [F], F32, kind="ExternalInput")
    b2 = nc.dram_tensor("b2", [D], F32, kind="ExternalInput")
    g1 = nc.dram_tensor("g1", [D], F32, kind="ExternalInput")
    be1 = nc.dram_tensor("be1", [D], F32, kind="ExternalInput")
    g2 = nc.dram_tensor("g2", [D], F32, kind="ExternalInput")
    be2 = nc.dram_tensor("be2", [D], F32, kind="ExternalInput")
    out = nc.dram_tensor("out", [Lq, D], F32, kind="ExternalOutput")
    import os
    dbg = None
    if os.environ.get("K_DBG"):
        dbg = nc.dram_tensor("dbg", [LT, P, D], F32, kind="ExternalOutput")

    def bcast(dram_vec):
        a = dram_vec[:]
        return bass.AP(tensor=a.tensor, offset=a.offset, ap=[[0, P]] + a.ap)

    with tile.TileContext(nc) as tc:
        with (
            tc.tile_pool(name="persist", bufs=1) as persist,
            tc.tile_pool(name="small", bufs=8) as small,
            tc.tile_pool(name="w1p", bufs=W1_PREF) as w1p,
        ):
            ident16 = persist.tile([P, P], BF16)
            make_identity(nc, ident16)
            b1s = persist.tile([P, FT], F32)
            nc.sync.dma_start(out=b1s, in_=b1[:].rearrange("(t p) -> p t", p=P))
            b2s = persist.tile([P, DT], F32)
            nc.sync.dma_start(out=b2s, in_=b2[:].rearrange("(t p) -> p t", p=P))
            epst = persist.tile([P, 1], F32)
            nc.vector.memset(epst, LN_EPS)
            esht = persist.tile([P, 1], F32)
            nc.vector.memset(esht, -ESHIFT)
            zer65 = persist.tile([P, W], BF16)
            nc.vector.memset(zer65, 0.0)
            # new_x starts as the fp32 query rows (the attention residual),
            # accumulates the attention output, and after LN1 is reused as
            # the residual-2 accumulator.
            new_x = persist.tile([P, LT, D], F32)
            for lt in range(LT):
                nc.sync.dma_start(out=new_x[:, lt, :], in_=xq32_d[lt])

            # ---------------- attention ----------------
            with (
                tc.tile_pool(name="attn_sb", bufs=1) as asb,
                tc.tile_pool(name="etp", bufs=1) as etp,
                tc.tile_pool(name="utsp", bufs=4) as utsp,
            ):
                xTs = asb.tile([P, DT, S], BF16)
                for t in range(DT):
                    nc.sync.dma_start(out=xTs[:, t, :], in_=xT_d[t])
                v16s = asb.tile([P, ST // 2, H, W], BF16)
                for u in range(ST // 2):
                    nc.sync.dma_start(
                        out=v16s[:, u, :, :],
                        in_=v16_d[u].rearrange("p (h w) -> p h w", w=W))
                v8s = asb.tile([P, ST // 2, H, W], FP8)
                for u in range(ST // 2):
                    nc.sync.dma_start(
                        out=v8s[:, u, :, :],
                        in_=v8_d[u].rearrange("p (h w) -> p h w", w=W))

                # prefetch the first w1 f-tiles so FFN1 starts instantly
                w1tiles = []
                for ft in range(W1_PREF):
                    wt = w1p.tile([P, DT, P], BF16, tag="w1t")
                    nc.sync.dma_start(out=wt, in_=w1t[ft].rearrange(
                        "p (t m) -> p t m", m=P))
                    w1tiles.append(wt)

                with (
                    tc.tile_pool(name="scp", bufs=1, space="PSUM") as scp,
                    tc.tile_pool(name="utp", bufs=2, space="PSUM") as utp,
                    tc.tile_pool(name="upp", bufs=2, space="PSUM") as upp,
                ):
                    # PE warmup: ~10us of back-to-back matmuls (gated on the
                    # first xT DMA) flips the HAM clock gate to 8/8 just as
                    # the first scores issue
                    wsc = scp.tile([P, SL], F32, tag="sc1", bufs=2)
                    for _ in range(46):
                        nc.tensor.matmul(
                            wsc, xTs[:, 0, 0:P], xTs[:, 0, 0:SL],
                            start=True, stop=True)

                    ets_cur = {}    # (h01, u2) -> et tile consumed this slot
                    ek = [0]        # cross-half exp slab round-robin counter

                    def emit_exp(sc, dst, fp8, has_diag):
                        if has_diag:
                            # diagonal weights dominate the softmax; keep
                            # their exp at full ACT accuracy
                            nc.scalar.activation(
                                out=dst, in_=sc, func=EXP,
                                scale=1.0 / 8.0, bias=esht)
                        elif not fp8:
                            nc.vector.tensor_scalar(
                                out=dst.bitcast(I16), in0=sc,
                                scalar1=FEXP_A16, scalar2=FEXP_B16,
                                op0=MUL, op1=ADD)
                        else:
                            k = ek[0] % 32
                            ek[0] += 1
                            if k % 3 == 1:
                                nc.vector.tensor_scalar(
                                    out=dst.bitcast(U8), in0=sc,
                                    scalar1=FEXP_A8, scalar2=FEXP_B8,
                                    op0=MUL, op1=ADD)
                            else:
                                nc.scalar.activation(
                                    out=dst, in_=sc, func=EXP,
                                    scale=1.0 / 8.0, bias=esht)

                    epi_prev = None   # (pair, uts dict) awaiting epilogue
                    for slot in range(-1, NP + 1):
                        pN = slot + 1   # pair whose scores/exp run this slot
                        pA = slot       # pair whose AV (both slabs) runs now
                        ets_new = {}
                        uts_all = {}

                        def av_step(u2, s, uls):
                            for h01 in range(2):
                                if u2 < NU2 // 2:
                                    for j in range(2):
                                        nc.tensor.matmul(
                                            uls[h01],
                                            v16s[:, 2 * u2 + j,
                                                 2 * pA + h01, :],
                                            ets_cur[(h01, u2)][
                                                :, j, s * SL:(s + 1) * SL],
                                            start=(u2 == 0 and j == 0),
                                            stop=False)
                                    continue
                                nc.tensor.matmul(
                                    uls[h01],
                                    v8s[:, 2 * (u2 - NU2 // 2):
                                        2 * (u2 - NU2 // 2) + 2,
                                        2 * pA + h01, :],
                                    ets_cur[(h01, u2)][:, :,
                                                       s * SL:(s + 1) * SL],
                                    start=False,
                                    stop=(u2 == NU2 - 1),
                                    perf_mode=DR)

                        # epilogue of the PREVIOUS pair, broken into small
                        # ops and spread across this slot so the DVE FIFO
                        # never blocks the exp pipeline for long
                        epi_ops = []
                        if epi_prev is not None:
                            pE, utsE = epi_prev
                            for h01 in range(2):
                                g = 2 * pE + h01
                                up = upp.tile([P, LT, W + 3], BF16,
                                              tag="up", bufs=1,
                                              name=f"up{slot}{h01}")
                                rz8 = small.tile([P, LT], F32, tag="rz8",
                                                 name=f"rz{slot}{h01}")

                                def mk_t(h01, lt, up=up):
                                    def go():
                                        src_ = utsE[(h01, lt // (LT // 2))]
                                        lo = (lt % (LT // 2)) * P
                                        nc.tensor.transpose(
                                            up[:, lt, 0:W],
                                            src_[:, lo:lo + P],
                                            ident16[0:W, 0:W])
                                    return go

                                def mk_r(up=up, rz8=rz8):
                                    def go():
                                        nc.vector.reciprocal(
                                            out=rz8, in_=up[:, :, E])
                                    return go

                                def mk_s(lt, g=g, up=up, rz8=rz8):
                                    def go():
                                        nc.vector.scalar_tensor_tensor(
                                            out=new_x[:, lt,
                                                      g * E:(g + 1) * E],
                                            in0=up[:, lt, 0:E],
                                            scalar=rz8[:, lt:lt + 1],
                                            in1=new_x[:, lt,
                                                      g * E:(g + 1) * E],
                                            op0=MUL, op1=ADD)
                                    return go
                                for lt in range(LT):
                                    epi_ops.append(mk_t(h01, lt))
                                epi_ops.append(mk_r())
                                for lt in range(LT):
                                    epi_ops.append(mk_s(lt))

                        if 0 <= pA < NP:
                            uls = [utp.tile([W, SL], F32, tag="ut",
                                            name=f"ut{slot}a{i}")
                                   for i in range(2)]
                        for u2 in range(NU2):
                            if 0 <= pA < NP and u2 == NU2 // 2:
                                # slab 0 done: drain it and switch the
                                # accumulators to slab 1
                                for h01 in range(2):
                                    uts = utsp.tile([W, SL], BF16,
                                                    tag="uts", bufs=8)
                                    nc.vector.tensor_copy(out=uts,
                                                          in_=uls[h01])
                                    uts_all[(h01, 0)] = uts
                                uls = [utp.tile([W, SL], F32, tag="ut",
                                                name=f"ut{slot}b{i}")
                                       for i in range(2)]
                            if pN < NP:
                                for j in range(2):
                                    u = 2 * u2 + j
                                    fp8 = u2 >= NU2 // 2
                                    for h01 in range(2):
                                        ro = h01 * E
                                        if j == 0:
                                            et = etp.tile(
                                                [P, 2, Lq],
                                                FP8 if fp8 else BF16,
                                                tag="et8" if fp8 else "et16",
                                                bufs=2 * ST // 2,
                                                name=f"et{slot}_{u2}_{h01}")
                                            ets_new[(h01, u2)] = et
                                        et = ets_new[(h01, u2)]
                                        for s in range(NSL):
                                            sc = scp.tile([P, SL], F32,
                                                          tag=f"sc{h01}",
                                                          bufs=3 - h01,
                                                          name=f"sc{h01}_{s}")
                                            nc.tensor.matmul(
                                                sc,
                                                xTs[ro:ro + E, pN,
                                                    u * P:(u + 1) * P],
                                                xTs[ro:ro + E, pN,
                                                    s * SL:(s + 1) * SL],
                                                start=True, stop=True)
                                            emit_exp(
                                                sc,
                                                et[:, j,
                                                   s * SL:(s + 1) * SL],
                                                fp8,
                                                (not fp8) and u // (NU2 // 2)
                                                == s)
                                    if 0 <= pA < NP and j == 0:
                                        # two AV steps of the previous pair
                                        # per iteration: slab 0 in the first
                                        # half of the slot, slab 1 after
                                        s_, base = ((0, 0) if u2 < NU2 // 2
                                                    else (1, NU2))
                                        av_step(2 * u2 - base, s_, uls)
                                        av_step(2 * u2 + 1 - base, s_, uls)
                            elif 0 <= pA < NP:
                                s_, base = ((0, 0) if u2 < NU2 // 2
                                            else (1, NU2))
                                av_step(2 * u2 - base, s_, uls)
                                av_step(2 * u2 + 1 - base, s_, uls)
                            # spread epilogue pops over the PE-sparse
                            # cross-half iterations
                            if u2 >= NU2 // 2 - 1:
                                for _ in range(7):
                                    if epi_ops:
                                        epi_ops.pop(0)()
                        while epi_ops:
                            epi_ops.pop(0)()
                        if 0 <= pA < NP:
                            for h01 in range(2):
                                uts = utsp.tile([W, SL], BF16, tag="uts",
                                                bufs=8)
                                nc.vector.tensor_copy(out=uts, in_=uls[h01])
                                uts_all[(h01, 1)] = uts
                            epi_prev = (pA, uts_all)
                        # the ets produced this slot become next slot's input
                        if pN < NP:
                            ets_cur = ets_new

            if dbg is not None:
                for lt in range(LT):
                    nc.sync.dma_start(out=dbg[lt], in_=new_x[:, lt, :])
            # ---------------- LN1 + FFN ----------------
            with (
                tc.tile_pool(name="ffn_sb", bufs=1) as fsb,
                tc.tile_pool(name="w2p", bufs=3) as w2p,
                tc.tile_pool(name="ysbp", bufs=2) as ysbp,
                tc.tile_pool(name="outp", bufs=2) as outp,
            ):
                # residual-1 complete in new_x; LN1 -> x1b (bf16).
                x1b = fsb.tile([P, LT, D], BF16)
                gb1 = beb1 = None
                if affine1:
                    gb1 = fsb.tile([P, D], F32)
                    nc.gpsimd.dma_start(out=gb1, in_=bcast(g1))
                    beb1 = fsb.tile([P, D], F32)
                    nc.gpsimd.dma_start(out=beb1, in_=bcast(be1))
                for lt in range(LT):
                    _layer_norm(nc, small, x1b[:, lt, :], new_x[:, lt, :],
                                gb1, beb1, epst, GS, affine1,
                                on_dve=(lt % 2 == 1))

                # x1 transposed to [d, l] for the FFN
                x1T = fsb.tile([P, DT, Lq], BF16)
                with tc.tile_pool(name="x1tp", bufs=4, space="PSUM") as x1tp:
                    for lt in range(LT):
                        for c in range(DT):
                            tp = x1tp.tile([P, P], BF16, tag="tp",
                                           name=f"tp{lt}{c}")
                            nc.tensor.transpose(
                                tp, x1b[:, lt, c * P:(c + 1) * P], ident16)
                            nc.vector.tensor_copy(
                                out=x1T[:, c, lt * P:(lt + 1) * P], in_=tp)

                hts = hts16 = None
                if NF8:
                    hts = fsb.tile([P, NF8, Lq], FP8)
                if NF8 < FT:
                    hts16 = fsb.tile([P, FT - NF8, Lq], BF16)
                with tc.tile_pool(name="hpp", bufs=4, space="PSUM") as hpp:
                    for ft in range(FT):
                        if ft < W1_PREF:
                            wt = w1tiles[ft]
                        else:
                            wt = w1p.tile([P, DT, P], BF16, tag="w1t")
                            nc.sync.dma_start(
                                out=wt, in_=w1t[ft].rearrange(
                                    "p (t m) -> p t m", m=P))
                        hps = []
                        for _s in range(NSL):
                            hp = hpp.tile([P, SL], F32, tag="hp")
                            hps.append(hp)
                        for dc in range(DT):
                            for s in range(NSL):
                                nc.tensor.matmul(
                                    hps[s], wt[:, dc, :],
                                    x1T[:, dc, s * SL:(s + 1) * SL],
                                    start=(dc == 0), stop=(dc == DT - 1))
                        hdst = (hts[:, ft, :] if ft < NF8
                                else hts16[:, ft - NF8, :])
                        for s in range(NSL):
                            nc.scalar.activation(
                                out=hdst[:, s * SL:(s + 1) * SL],
                                in_=hps[s], func=RELU,
                                bias=b1s[:, ft:ft + 1])

                gb2 = beb2 = None
                if affine2:
                    gb2 = fsb.tile([P, D], F32)
                    nc.gpsimd.dma_start(out=gb2, in_=bcast(g2))
                    beb2 = fsb.tile([P, D], F32)
                    nc.gpsimd.dma_start(out=beb2, in_=bcast(be2))

                with (
                    tc.tile_pool(name="ypp", bufs=4, space="PSUM") as ypp,
                    tc.tile_pool(name="tpp", bufs=4, space="PSUM") as tpp,
                ):
                    for dt in range(DT):
                        w2s = w2s16 = None
                        if NF8:
                            w2s = w2p.tile([P, NF8 // 2, 2, P], FP8,
                                           tag="w2t")
                            nc.sync.dma_start(
                                out=w2s, in_=w2t8[dt].rearrange(
                                    "p (a b m) -> p a b m", b=2, m=P))
                        if NF8 < FT:
                            w2s16 = w2p.tile([P, FT - NF8, P], BF16,
                                             tag="w2t16")
                            nc.sync.dma_start(
                                out=w2s16, in_=w2t16[dt].rearrange(
                                    "p (t m) -> p t m", m=P))
                        yps = []
                        for _s in range(NSL):
                            yp = ypp.tile([P, SL], F32, tag="yp")
                            yps.append(yp)
                        for ft2 in range(NF8 // 2):
                            for s in range(NSL):
                                nc.tensor.matmul(
                                    yps[s], w2s[:, ft2, :, :],
                                    hts[:, 2 * ft2:2 * ft2 + 2,
                                        s * SL:(s + 1) * SL],
                                    start=(ft2 == 0),
                                    stop=(NF8 == FT and ft2 == NF8 // 2 - 1),
                                    perf_mode=DR)
                        for ft in range(NF8, FT):
                            for s in range(NSL):
                                nc.tensor.matmul(
                                    yps[s], w2s16[:, ft - NF8, :],
                                    hts16[:, ft - NF8,
                                          s * SL:(s + 1) * SL],
                                    start=(ft == 0), stop=(ft == FT - 1))
                        ysb = ysbp.tile([P, Lq], BF16, tag="ysb")
                        for s in range(NSL):
                            nc.vector.tensor_scalar(
                                out=ysb[:, s * SL:(s + 1) * SL], in0=yps[s],
                                scalar1=1.0 / S2,
                                scalar2=b2s[:, dt:dt + 1],
                                op0=MUL, op1=ADD)
                        # transpose y back to [l, d] and add the x1 residual
                        for lt in range(LT):
                            tp = tpp.tile([P, P], BF16)
                            nc.tensor.transpose(
                                tp, ysb[:, lt * P:(lt + 1) * P], ident16)
                            nc.vector.scalar_tensor_tensor(
                                out=new_x[:, lt, dt * P:(dt + 1) * P],
                                in0=tp, scalar=1.0,
                                in1=x1b[:, lt, dt * P:(dt + 1) * P],
                                op0=MUL, op1=ADD)

                    for lt in range(LT):
                        ot = outp.tile([P, D], F32, tag="ot")
                        _layer_norm(nc, small, ot, new_x[:, lt, :],
                                    gb2, beb2, epst, GS, affine2,
                                    on_dve=(lt % 2 == 1))
                        nc.sync.dma_start(
                            out=out[lt * P:(lt + 1) * P, 0:D // 2],
                            in_=ot[:, 0:D // 2])
                        nc.sync.dma_start(
                            out=out[lt * P:(lt + 1) * P, D // 2:D],
                            in_=ot[:, D // 2:D])

    nc.finalize()
    return nc


def _layer_norm(nc, small, out_ap, x_ap, gb, beb, epst, GS, affine,
                on_dve=False):
    """out = (x - mean(x)) * rsqrt(var(x) + eps) [* g + be] over free dim.
    The normalize pass runs on ACT by default, or DVE (on_dve) so
    consecutive LNs can alternate engines."""
    D = x_ap.shape[-1]
    ngr = D // GS
    st = small.tile([P, ngr, 6], F32, tag="bnst")
    xg = x_ap.rearrange("p (g k) -> p g k", k=GS)
    for g in range(ngr):
        nc.vector.bn_stats(out=st[:, g, :], in_=xg[:, g, :])
    mv = small.tile([P, 2], F32, tag="bnmv")
    nc.vector.bn_aggr(out=mv, in_=st)
    sd = small.tile([P, 1], F32, tag="sd")
    nc.scalar.activation(out=sd, in_=mv[:, 1:2], func=SQRT, bias=epst)
    rstd = small.tile([P, 1], F32, tag="rstd")
    nc.vector.reciprocal(out=rstd, in_=sd)
    dst = out_ap
    if affine:
        dst = small.tile([P, D], F32, tag="xn", bufs=2)
    if on_dve:
        nc.vector.tensor_scalar(
            out=dst, in0=x_ap, scalar1=mv[:, 0:1], scalar2=rstd,
            op0=mybir.AluOpType.subtract, op1=MUL)
    else:
        nmr = small.tile([P, 1], F32, tag="nmr")
        nc.vector.scalar_tensor_tensor(
            out=nmr, in0=mv[:, 0:1], scalar=-1.0, in1=rstd,
            op0=MUL, op1=MUL)
        nc.scalar.activation(out=dst, in_=x_ap, func=IDENT,
                             bias=nmr, scale=rstd)
    if affine:
        nc.vector.tensor_mul(out=dst, in0=dst, in1=gb)
        nc.vector.tensor_add(out=out_ap, in0=dst, in1=beb)


# ---------------------------------------------------------------------------
# host side
# ---------------------------------------------------------------------------

_PROG_CACHE = {}


def get_program(S=2048, D=1024, F=4096, affine1=False, affine2=False):
    key = (S, D, F, affine1, affine2)
    if key not in _PROG_CACHE:
        _PROG_CACHE[key] = build_program(S, D, F, affine1, affine2)
    return _PROG_CACHE[key]


def make_in_maps(x, w1, b1, w2, b2, g1, be1, g2, be2, n_cores=8):
    import ml_dtypes
    FP8NP = ml_dtypes.float8_e4m3fn
    B, L, D = x.shape
    F = w1.shape[0]
    S = L
    H = D // E
    Lq = L // 2
    ST, LT, DT, FT = S // P, Lq // P, D // P, F // P
    x = np.asarray(x, np.float32)
    # w1t[ft, p, dc*128+m] = w1[ft*128+m, dc*128+p]
    w1t = np.ascontiguousarray(
        w1.astype(np.float32).reshape(FT, P, DT, P).transpose(0, 3, 2, 1)
        .reshape(FT, P, D)).astype(ml_dtypes.bfloat16)
    # w2 transposed [dt, p, ft*128+m] = w2[dt*128+m, ft*128+p] * S2,
    # first NF8 f-tiles in fp8 (DoubleRow), the rest in bf16 (the *32
    # scale is exact in bf16 and lets one uniform unscale serve both)
    NF8 = 2 * NFP8_FT2 if FFN2_FP8 else 0
    w2m = (w2.astype(np.float32).reshape(DT, P, FT, P)
           .transpose(0, 3, 2, 1).reshape(DT, P, FT, P)) * S2
    common = dict(w1t=w1t, b1=b1, b2=b2, g1=g1, be1=be1, g2=g2, be2=be2)
    if NF8:
        common["w2t8"] = np.ascontiguousarray(
            w2m[:, :, :NF8]).astype(FP8NP).reshape(DT, P, NF8 * P)
    if NF8 < FT:
        common["w2t16"] = np.ascontiguousarray(
            w2m[:, :, NF8:]).astype(ml_dtypes.bfloat16).reshape(
                DT, P, (FT - NF8) * P)
    in_maps = []
    for c in range(n_cores):
        b, half = c // 2, c % 2
        lo = half * Lq
        xq = x[b, lo:lo + Lq]
        xo = x[b, Lq - lo:2 * Lq - lo]
        xbl = np.concatenate([xq, xo], axis=0)
        # xT[t, p, s] = xbl[s, t*128+p]
        xT = np.ascontiguousarray(
            xbl.reshape(S, DT, P).transpose(1, 2, 0)).astype(
                ml_dtypes.bfloat16)
        # v[u, p, h, :] = [xbl[u*128+p, h*64:(h+1)*64] | 1.0]
        # same-half key tiles (u < ST/2, contain the diagonal) in bf16,
        # cross-half tiles in fp8.
        va = np.ones((ST, P, H, W), np.float32)
        va[:, :, :, 0:E] = xbl.reshape(ST, P, H, E)
        v16 = np.ascontiguousarray(va[:ST // 2]).astype(
            ml_dtypes.bfloat16).reshape(ST // 2, P, H * W)
        v8 = np.ascontiguousarray(va[ST // 2:]).astype(FP8NP).reshape(
            ST // 2, P, H * W)
        xq32 = np.ascontiguousarray(xbl[0:Lq].reshape(LT, P, D))
        in_maps.append(dict(xT=xT, v16=v16, v8=v8, xq32=xq32, **common))
    return in_maps


def kernel(x, w1, b1, w2, b2, g1, be1, g2, be2):
    from concourse.bass_utils import run_bass_kernel_spmd

    x = np.asarray(x, dtype=np.float32)
    B, L, D = x.shape
    F = w1.shape[0]
    Lq = L // 2
    n_cores = 2 * B
    g1 = np.asarray(g1, np.float32)
    be1 = np.asarray(be1, np.float32)
    g2 = np.asarray(g2, np.float32)
    be2 = np.asarray(be2, np.float32)
    affine1 = not (np.all(g1 == 1.0) and np.all(be1 == 0.0))
    affine2 = not (np.all(g2 == 1.0) and np.all(be2 == 0.0))
    nc = get_program(L, D, F, affine1, affine2)
    in_maps = make_in_maps(x, np.asarray(w1, np.float32),
                           np.asarray(b1, np.float32),
                           np.asarray(w2, np.float32),
                           np.asarray(b2, np.float32),
                           g1, be1, g2, be2, n_cores)
    res = run_bass_kernel_spmd(nc, in_maps, core_ids=list(range(n_cores)))
    outp = np.empty((B, L, D), dtype=np.float32)
    for c in range(n_cores):
        b, half = c // 2, c % 2
        outp[b, half * Lq:(half + 1) * Lq] = res.results[c]["out"]
    return outp


# revision 34
# speedup vs baseline: 1.2034x; 1.0361x over previous
"""Trainium2 Bass kernel for a dense transformer encoder layer.

Reference computation (per batch b):
    q = x.reshape(L, H, E)                       # H=16 heads, E=64
    scores = q @ q^T per head, scaled softmax    # A = softmax(s/8)
    new_x  = concat_h(A_h @ q_h)                 # [L, D]
    x1 = LN(x + new_x; g1, be1)
    y  = relu(x1 @ w1^T + b1) @ w2^T + b2
    out = LN(x1 + y; g2, be2)

Sharding: pure data parallel over (batch, seq-half): core c handles
batch c//2, query rows [(c%2)*1024, +1024).  Keys span the full sequence
of that batch, so every core gets the whole x[b] (queries reordered
first) and the full FFN weights.  No device collectives.

Per-core schedule:
  - All big attention operands are pre-laid-out on the HOST: x^T tiles
    ([d, s] bf16 for the scores matmuls), the fp8 [V|1] array (value
    rows interleaved with a ones column per head, for the AV matmuls),
    and an fp32 copy of the query rows that seeds the attention
    residual accumulator.  This removes all on-device layout work
    (transposes + copies) from the attention phase.
  - Heads are processed in PAIRS: head 2t lives in PE rows 0-63 and
    head 2t+1 in rows 64-127 of the same x^T d-tile, so their scores
    matmuls (K=64 contraction) run CONCURRENTLY in separate PE
    row-groups (tile_position row tiling).
  - exp(s/8 - 1) is written directly in fp8-e4m3: ACT exp for most
    tiles, and a Schraudolph fast-exp on the DVE (affine int ->
    uint8 bitcast IS e4m3 exp, with negative results clamped to +0 by
    the uint8 saturation) for the rest -- the split keeps both engines
    busy since exp throughput paces the whole attention phase.
  - AV matmuls run in fp8 DoubleRow mode: 256-key contraction per
    instruction (the [V|1] array pairs adjacent key tiles in its free
    dim), halving PE time.  The ones column emits the softmax
    denominator in row 64 of U^T = [V|1]^T E^T.
  - U^T tiles are PE-transposed back to [l, 65] (bf16); one batched
    reciprocal per head and one fused scalar_tensor_tensor per
    (head, l-tile) divide by the denominator and add the fp32 x
    residual in place.
  - LayerNorm = bn_stats/bn_aggr + sqrt + one ACT Identity pass.
  - FFN1 in bf16 (streamed weights, prefetched during attention);
    FFN2 in fp8 DoubleRow with weights scaled x32 on the host (the
    1/32 folds into the bias pass).
"""

import numpy as np

import concourse.bass as bass
import concourse.tile as tile
from concourse import bacc
from concourse import mybir
from concourse.masks import make_identity

F32 = mybir.dt.float32
BF16 = mybir.dt.bfloat16
FP8 = mybir.dt.float8e4
U8 = mybir.dt.uint8
EXP = mybir.ActivationFunctionType.Exp
RELU = mybir.ActivationFunctionType.Relu
SQRT = mybir.ActivationFunctionType.Sqrt
IDENT = mybir.ActivationFunctionType.Identity
ADD = mybir.AluOpType.add
MUL = mybir.AluOpType.mult
DR = mybir.MatmulPerfMode.DoubleRow

LN_EPS = 1e-5
ESHIFT = 3.5    # logit shift: et = exp(s/8 - ESHIFT); keeps fp8 et < 240
LOG2E = float(np.log2(np.e))
# Schraudolph fast-exp in e4m3 bits: et = bitcast_fp8(uint8(A*s + B));
# uint8 saturation clamps negative results (tiny et) to +0.  Only used on
# cross-half key tiles: no diagonal, so the affine result stays in [0, 110].
FEXP_A8 = LOG2E                      # d(bits)/ds = 8 * log2e / 8
FEXP_B8 = float(8 * 7 - 0.5 + 0.344 - 8 * LOG2E * ESHIFT)
# Schraudolph in bf16 bits via int16: d(bits)/ds = 2^7 * log2e / 8
FEXP_A16 = float(2 ** 7 * LOG2E / 8.0)
FEXP_B16 = float(127 * 2 ** 7 - 5.6 - 2 ** 7 * LOG2E * ESHIFT)
I16 = mybir.dt.int16
E = 64          # head dim
W = E + 1       # head dim + ones column
P = 128         # partitions
MDVE32 = 11     # of the 32 cross-half exp slabs per pair, DVE takes this many
FFN2_FP8 = True
NFP8_FT2 = 8    # f-tile pairs of the FFN2 contraction done in fp8 DoubleRow
S2 = 32.0       # host-side scale on w2 in fp8 (folded out in the bias pass)


def build_program(S=2048, D=1024, F=4096, affine1=False, affine2=False):
    """Build the per-core Bass program.  S = full seq len; queries are the
    first Lq = S//2 rows of the reordered sequence."""
    H = D // E
    NP = H // 2          # head pairs
    Lq = S // 2
    ST = S // P          # key tiles
    NU2 = ST // 2        # key tile pairs (DoubleRow contracts 2 at a time)
    LT = Lq // P         # query row tiles
    DT = D // P          # d chunks
    FT = F // P          # f tiles
    SL = 512             # matmul moving slab (one PSUM bank of fp32)
    NSL = Lq // SL
    GS = min(512, D)     # bn_stats subgroup size
    W1_PREF = 6          # w1 f-tiles prefetched during attention

    nc = bacc.Bacc("TRN2")

    xT_d = nc.dram_tensor("xT", [DT, P, S], BF16, kind="ExternalInput")
    xT2_d = nc.dram_tensor("xT2", [DT, P, S], BF16, kind="ExternalInput")
    v8_d = nc.dram_tensor("v8", [ST // 2, P, H * W], FP8,
                          kind="ExternalInput")
    v16_d = nc.dram_tensor("v16", [ST // 2, P, H * W], BF16,
                           kind="ExternalInput")
    xq32_d = nc.dram_tensor("xq32", [LT, P, D], F32, kind="ExternalInput")
    w1t = nc.dram_tensor("w1t", [FT, P, D], BF16, kind="ExternalInput")
    NF8 = 2 * NFP8_FT2 if FFN2_FP8 else 0   # f-tiles of FFN2 in fp8
    w2t8 = w2t16 = None
    if NF8:
        w2t8 = nc.dram_tensor("w2t8", [DT, P, NF8 * P], FP8,
                              kind="ExternalInput")
    if NF8 < FT:
        w2t16 = nc.dram_tensor("w2t16", [DT, P, (FT - NF8) * P], BF16,
                               kind="ExternalInput")
    b1 = nc.dram_tensor("b1", [F], F32, kind="ExternalInput")
    b2 = nc.dram_tensor("b2", [D], F32, kind="ExternalInput")
    g1 = nc.dram_tensor("g1", [D], F32, kind="ExternalInput")
    be1 = nc.dram_tensor("be1", [D], F32, kind="ExternalInput")
    g2 = nc.dram_tensor("g2", [D], F32, kind="ExternalInput")
    be2 = nc.dram_tensor("be2", [D], F32, kind="ExternalInput")
    out = nc.dram_tensor("out", [Lq, D], F32, kind="ExternalOutput")
    import os
    dbg = None
    if os.environ.get("K_DBG"):
        dbg = nc.dram_tensor("dbg", [LT, P, D], F32, kind="ExternalOutput")

    def bcast(dram_vec):
        a = dram_vec[:]
        return bass.AP(tensor=a.tensor, offset=a.offset, ap=[[0, P]] + a.ap)

    with tile.TileContext(nc) as tc:
        with (
            tc.tile_pool(name="persist", bufs=1) as persist,
            tc.tile_pool(name="small", bufs=8) as small,
            tc.tile_pool(name="w1p", bufs=W1_PREF) as w1p,
        ):
            ident16 = persist.tile([P, P], BF16)
            make_identity(nc, ident16)
            b1s = persist.tile([P, FT], F32)
            nc.sync.dma_start(out=b1s, in_=b1[:].rearrange("(t p) -> p t", p=P))
            b2s = persist.tile([P, DT], F32)
            nc.sync.dma_start(out=b2s, in_=b2[:].rearrange("(t p) -> p t", p=P))
            epst = persist.tile([P, 1], F32)
            nc.vector.memset(epst, LN_EPS)
            esht = persist.tile([P, 1], F32)
            nc.vector.memset(esht, -ESHIFT)
            zer65 = persist.tile([P, W], BF16)
            nc.vector.memset(zer65, 0.0)
            # new_x starts as the fp32 query rows (the attention residual),
            # accumulates the attention output, and after LN1 is reused as
            # the residual-2 accumulator.
            new_x = persist.tile([P, LT, D], F32)
            for lt in range(LT):
                nc.sync.dma_start(out=new_x[:, lt, :], in_=xq32_d[lt])

            # ---------------- attention ----------------
            with (
                tc.tile_pool(name="attn_sb", bufs=1) as asb,
                tc.tile_pool(name="etp", bufs=1) as etp,
                tc.tile_pool(name="utsp", bufs=4) as utsp,
            ):
                xTs = asb.tile([P, DT, S], BF16)
                for t in range(DT):
                    nc.sync.dma_start(out=xTs[:, t, :], in_=xT_d[t])
                xT2s = asb.tile([P, DT, S], BF16)
                for t in range(DT):
                    nc.sync.dma_start(out=xT2s[:, t, :], in_=xT2_d[t])
                v16s = asb.tile([P, ST // 2, H, W], BF16)
                for u in range(ST // 2):
                    nc.sync.dma_start(
                        out=v16s[:, u, :, :],
                        in_=v16_d[u].rearrange("p (h w) -> p h w", w=W))
                v8s = asb.tile([P, ST // 2, H, W], FP8)
                for u in range(ST // 2):
                    nc.sync.dma_start(
                        out=v8s[:, u, :, :],
                        in_=v8_d[u].rearrange("p (h w) -> p h w", w=W))

                # prefetch the first w1 f-tiles so FFN1 starts instantly
                w1tiles = []
                for ft in range(W1_PREF):
                    wt = w1p.tile([P, DT, P], BF16, tag="w1t")
                    nc.sync.dma_start(out=wt, in_=w1t[ft].rearrange(
                        "p (t m) -> p t m", m=P))
                    w1tiles.append(wt)

                with (
                    tc.tile_pool(name="scp", bufs=1, space="PSUM") as scp,
                    tc.tile_pool(name="utp", bufs=2, space="PSUM") as utp,
                    tc.tile_pool(name="upp", bufs=2, space="PSUM") as upp,
                ):
                    # PE warmup: ~10us of back-to-back matmuls (gated on the
                    # first xT DMA) flips the HAM clock gate to 8/8 just as
                    # the first scores issue
                    wsc = scp.tile([P, SL], F32, tag="sc", bufs=5)
                    for _ in range(46):
                        nc.tensor.matmul(
                            wsc, xTs[:, 0, 0:P], xTs[:, 0, 0:SL],
                            start=True, stop=True)

                    ets_cur = {}    # (h01, u2) -> et tile consumed this slot
                    ek = [0]        # cross-half exp slab round-robin counter

                    def emit_exp(sc, dst, fp8, has_diag):
                        if has_diag:
                            # diagonal weights dominate the softmax; keep
                            # their exp at full ACT accuracy
                            nc.scalar.activation(
                                out=dst, in_=sc, func=EXP,
                                scale=1.0 / 8.0, bias=esht)
                        elif not fp8:
                            nc.vector.tensor_scalar(
                                out=dst.bitcast(I16), in0=sc,
                                scalar1=FEXP_A16, scalar2=FEXP_B16,
                                op0=MUL, op1=ADD)
                        else:
                            k = ek[0] % 32
                            ek[0] += 1
                            if k % 3 == 1:
                                nc.vector.tensor_scalar(
                                    out=dst.bitcast(U8), in0=sc,
                                    scalar1=FEXP_A8, scalar2=FEXP_B8,
                                    op0=MUL, op1=ADD)
                            else:
                                nc.scalar.activation(
                                    out=dst, in_=sc, func=EXP,
                                    scale=1.0 / 8.0, bias=esht)

                    epi_prev = None   # (head, uts dict) awaiting epilogue
                    for slot in range(-1, H + 1):
                        hN = slot + 1   # head whose scores/exp run this slot
                        hA = slot       # head whose AV (both slabs) runs now
                        ets_new = {}
                        uts_all = {}
                        roN = (hN % 2) * E
                        roA = (hA % 2) * E

                        def av_step(u2, s, uls):
                            if u2 < NU2 // 2:
                                for j in range(2):
                                    nc.tensor.matmul(
                                        uls[0],
                                        v16s[:, 2 * u2 + j, hA, :],
                                        ets_cur[u2][:, j,
                                                    s * SL:(s + 1) * SL],
                                        start=(u2 == 0 and j == 0),
                                        stop=False)
                                return
                            nc.tensor.matmul(
                                uls[0],
                                v8s[:, 2 * (u2 - NU2 // 2):
                                    2 * (u2 - NU2 // 2) + 2, hA, :],
                                ets_cur[u2][:, :, s * SL:(s + 1) * SL],
                                start=False,
                                stop=(u2 == NU2 - 1),
                                perf_mode=DR)

                        # epilogue of the PREVIOUS head, broken into small
                        # ops and spread across this slot so the DVE FIFO
                        # never blocks the exp pipeline for long
                        epi_ops = []
                        if epi_prev is not None:
                            hE, utsE = epi_prev
                            up = upp.tile([P, LT, W + 3], BF16,
                                          tag="up", bufs=1,
                                          name=f"up{slot}")
                            rz8 = small.tile([P, LT], F32, tag="rz8",
                                             name=f"rz{slot}")

                            def mk_t(lt, up=up, utsE=utsE):
                                def go():
                                    src_ = utsE[lt // (LT // 2)]
                                    lo = (lt % (LT // 2)) * P
                                    nc.tensor.transpose(
                                        up[:, lt, 0:W],
                                        src_[:, lo:lo + P],
                                        ident16[0:W, 0:W])
                                return go

                            def mk_r(up=up, rz8=rz8):
                                def go():
                                    nc.vector.reciprocal(
                                        out=rz8, in_=up[:, :, E])
                                return go

                            def mk_s(lt, g=hE, up=up, rz8=rz8):
                                def go():
                                    nc.vector.scalar_tensor_tensor(
                                        out=new_x[:, lt, g * E:(g + 1) * E],
                                        in0=up[:, lt, 0:E],
                                        scalar=rz8[:, lt:lt + 1],
                                        in1=new_x[:, lt, g * E:(g + 1) * E],
                                        op0=MUL, op1=ADD)
                                return go
                            for lt in range(LT):
                                epi_ops.append(mk_t(lt))
                            epi_ops.append(mk_r())
                            for lt in range(LT):
                                epi_ops.append(mk_s(lt))

                        if 0 <= hA < H:
                            uls = [utp.tile([W, SL], F32, tag="ut",
                                            name=f"ut{slot}a")]
                        for u2 in range(NU2):
                            if 0 <= hA < H and u2 == NU2 // 2:
                                # slab 0 done: drain it and switch the
                                # accumulator to slab 1
                                uts = utsp.tile([W, SL], BF16,
                                                tag="uts", bufs=4)
                                nc.vector.tensor_copy(out=uts, in_=uls[0])
                                uts_all[0] = uts
                                uls = [utp.tile([W, SL], F32, tag="ut",
                                                name=f"ut{slot}b")]
                            if hN < H:
                                fp8 = u2 >= NU2 // 2
                                et = etp.tile(
                                    [P, 2, Lq],
                                    FP8 if fp8 else BF16,
                                    tag="et8" if fp8 else "et16",
                                    bufs=ST // 2,
                                    name=f"et{slot}_{u2}")
                                ets_new[u2] = et
                                for j in range(2):
                                    u = 2 * u2 + j
                                    xsrc = xTs if j == 0 else xT2s
                                    ro = roN if j == 0 else E - roN
                                    for s in range(NSL):
                                        sc = scp.tile([P, SL], F32,
                                                      tag="sc", bufs=5,
                                                      name=f"sc{u2}_{j}_{s}")
                                        nc.tensor.matmul(
                                            sc,
                                            xsrc[ro:ro + E, hN // 2,
                                                 u * P:(u + 1) * P],
                                            xsrc[ro:ro + E, hN // 2,
                                                 s * SL:(s + 1) * SL],
                                            start=True, stop=True)
                                        emit_exp(
                                            sc,
                                            et[:, j, s * SL:(s + 1) * SL],
                                            fp8,
                                            (not fp8) and u // (NU2 // 2)
                                            == s)
                                    if 0 <= hA < H and j == 0:
                                        # two AV steps of the previous head
                                        # per iteration: slab 0 in the first
                                        # half of the slot, slab 1 after
                                        s_, base = ((0, 0) if u2 < NU2 // 2
                                                    else (1, NU2))
                                        av_step(2 * u2 - base, s_, uls)
                                        av_step(2 * u2 + 1 - base, s_, uls)
                            elif 0 <= hA < H:
                                s_, base = ((0, 0) if u2 < NU2 // 2
                                            else (1, NU2))
                                av_step(2 * u2 - base, s_, uls)
                                av_step(2 * u2 + 1 - base, s_, uls)
                            # spread epilogue pops
                            for _ in range(3):
                                if epi_ops:
                                    epi_ops.pop(0)()
                        while epi_ops:
                            epi_ops.pop(0)()
                        if 0 <= hA < H:
                            uts = utsp.tile([W, SL], BF16, tag="uts",
                                            bufs=4)
                            nc.vector.tensor_copy(out=uts, in_=uls[0])
                            uts_all[1] = uts
                            epi_prev = (hA, uts_all)
                        # the ets produced this slot become next slot's input
                        if hN < H:
                            ets_cur = ets_new

            if dbg is not None:
                for lt in range(LT):
                    nc.sync.dma_start(out=dbg[lt], in_=new_x[:, lt, :])
            # ---------------- LN1 + FFN ----------------
            with (
                tc.tile_pool(name="ffn_sb", bufs=1) as fsb,
                tc.tile_pool(name="w2p", bufs=3) as w2p,
                tc.tile_pool(name="ysbp", bufs=2) as ysbp,
                tc.tile_pool(name="outp", bufs=2) as outp,
            ):
                # residual-1 complete in new_x; LN1 -> x1b (bf16).
                x1b = fsb.tile([P, LT, D], BF16)
                gb1 = beb1 = None
                if affine1:
                    gb1 = fsb.tile([P, D], F32)
                    nc.gpsimd.dma_start(out=gb1, in_=bcast(g1))
                    beb1 = fsb.tile([P, D], F32)
                    nc.gpsimd.dma_start(out=beb1, in_=bcast(be1))
                for lt in range(LT):
                    _layer_norm(nc, small, x1b[:, lt, :], new_x[:, lt, :],
                                gb1, beb1, epst, GS, affine1,
                                on_dve=(lt % 2 == 1))

                # x1 transposed to [d, l] for the FFN
                x1T = fsb.tile([P, DT, Lq], BF16)
                with tc.tile_pool(name="x1tp", bufs=4, space="PSUM") as x1tp:
                    for lt in range(LT):
                        for c in range(DT):
                            tp = x1tp.tile([P, P], BF16, tag="tp",
                                           name=f"tp{lt}{c}")
                            nc.tensor.transpose(
                                tp, x1b[:, lt, c * P:(c + 1) * P], ident16)
                            nc.vector.tensor_copy(
                                out=x1T[:, c, lt * P:(lt + 1) * P], in_=tp)

                hts = hts16 = None
                if NF8:
                    hts = fsb.tile([P, NF8, Lq], FP8)
                if NF8 < FT:
                    hts16 = fsb.tile([P, FT - NF8, Lq], BF16)
                with tc.tile_pool(name="hpp", bufs=4, space="PSUM") as hpp:
                    for ft in range(FT):
                        if ft < W1_PREF:
                            wt = w1tiles[ft]
                        else:
                            wt = w1p.tile([P, DT, P], BF16, tag="w1t")
                            nc.sync.dma_start(
                                out=wt, in_=w1t[ft].rearrange(
                                    "p (t m) -> p t m", m=P))
                        hps = []
                        for _s in range(NSL):
                            hp = hpp.tile([P, SL], F32, tag="hp")
                            hps.append(hp)
                        for dc in range(DT):
                            for s in range(NSL):
                                nc.tensor.matmul(
                                    hps[s], wt[:, dc, :],
                                    x1T[:, dc, s * SL:(s + 1) * SL],
                                    start=(dc == 0), stop=(dc == DT - 1))
                        hdst = (hts[:, ft, :] if ft < NF8
                                else hts16[:, ft - NF8, :])
                        for s in range(NSL):
                            nc.scalar.activation(
                                out=hdst[:, s * SL:(s + 1) * SL],
                                in_=hps[s], func=RELU,
                                bias=b1s[:, ft:ft + 1])

                gb2 = beb2 = None
                if affine2:
                    gb2 = fsb.tile([P, D], F32)
                    nc.gpsimd.dma_start(out=gb2, in_=bcast(g2))
                    beb2 = fsb.tile([P, D], F32)
                    nc.gpsimd.dma_start(out=beb2, in_=bcast(be2))

                with (
                    tc.tile_pool(name="ypp", bufs=4, space="PSUM") as ypp,
                    tc.tile_pool(name="tpp", bufs=4, space="PSUM") as tpp,
                ):
                    for dt in range(DT):
                        w2s = w2s16 = None
                        if NF8:
                            w2s = w2p.tile([P, NF8 // 2, 2, P], FP8,
                                           tag="w2t")
                            nc.sync.dma_start(
                                out=w2s, in_=w2t8[dt].rearrange(
                                    "p (a b m) -> p a b m", b=2, m=P))
                        if NF8 < FT:
                            w2s16 = w2p.tile([P, FT - NF8, P], BF16,
                                             tag="w2t16")
                            nc.sync.dma_start(
                                out=w2s16, in_=w2t16[dt].rearrange(
                                    "p (t m) -> p t m", m=P))
                        yps = []
                        for _s in range(NSL):
                            yp = ypp.tile([P, SL], F32, tag="yp")
                            yps.append(yp)
                        for ft2 in range(NF8 // 2):
                            for s in range(NSL):
                                nc.tensor.matmul(
                                    yps[s], w2s[:, ft2, :, :],
                                    hts[:, 2 * ft2:2 * ft2 + 2,
                                        s * SL:(s + 1) * SL],
                                    start=(ft2 == 0),
                                    stop=(NF8 == FT and ft2 == NF8 // 2 - 1),
                                    perf_mode=DR)
                        for ft in range(NF8, FT):
                            for s in range(NSL):
                                nc.tensor.matmul(
                                    yps[s], w2s16[:, ft - NF8, :],
                                    hts16[:, ft - NF8,
                                          s * SL:(s + 1) * SL],
                                    start=(ft == 0), stop=(ft == FT - 1))
                        ysb = ysbp.tile([P, Lq], BF16, tag="ysb")
                        for s in range(NSL):
                            nc.vector.tensor_scalar(
                                out=ysb[:, s * SL:(s + 1) * SL], in0=yps[s],
                                scalar1=1.0 / S2,
                                scalar2=b2s[:, dt:dt + 1],
                                op0=MUL, op1=ADD)
                        # transpose y back to [l, d] and add the x1 residual
                        for lt in range(LT):
                            tp = tpp.tile([P, P], BF16)
                            nc.tensor.transpose(
                                tp, ysb[:, lt * P:(lt + 1) * P], ident16)
                            nc.vector.scalar_tensor_tensor(
                                out=new_x[:, lt, dt * P:(dt + 1) * P],
                                in0=tp, scalar=1.0,
                                in1=x1b[:, lt, dt * P:(dt + 1) * P],
                                op0=MUL, op1=ADD)

                    for lt in range(LT):
                        ot = outp.tile([P, D], F32, tag="ot")
                        _layer_norm(nc, small, ot, new_x[:, lt, :],
                                    gb2, beb2, epst, GS, affine2,
                                    on_dve=(lt % 2 == 1))
                        nc.sync.dma_start(
                            out=out[lt * P:(lt + 1) * P, 0:D // 2],
                            in_=ot[:, 0:D // 2])
                        nc.sync.dma_start(
                            out=out[lt * P:(lt + 1) * P, D // 2:D],
                            in_=ot[:, D // 2:D])

    nc.finalize()
    return nc


def _layer_norm(nc, small, out_ap, x_ap, gb, beb, epst, GS, affine,
                on_dve=False):
    """out = (x - mean(x)) * rsqrt(var(x) + eps) [* g + be] over free dim.
    The normalize pass runs on ACT by default, or DVE (on_dve) so
    consecutive LNs can alternate engines."""
    D = x_ap.shape[-1]
    ngr = D // GS
    st = small.tile([P, ngr, 6], F32, tag="bnst")
    xg = x_ap.rearrange("p (g k) -> p g k", k=GS)
    for g in range(ngr):
        nc.vector.bn_stats(out=st[:, g, :], in_=xg[:, g, :])
    mv = small.tile([P, 2], F32, tag="bnmv")
    nc.vector.bn_aggr(out=mv, in_=st)
    sd = small.tile([P, 1], F32, tag="sd")
    nc.scalar.activation(out=sd, in_=mv[:, 1:2], func=SQRT, bias=epst)
    rstd = small.tile([P, 1], F32, tag="rstd")
    nc.vector.reciprocal(out=rstd, in_=sd)
    dst = out_ap
    if affine:
        dst = small.tile([P, D], F32, tag="xn", bufs=2)
    if on_dve:
        nc.vector.tensor_scalar(
            out=dst, in0=x_ap, scalar1=mv[:, 0:1], scalar2=rstd,
            op0=mybir.AluOpType.subtract, op1=MUL)
    else:
        nmr = small.tile([P, 1], F32, tag="nmr")
        nc.vector.scalar_tensor_tensor(
            out=nmr, in0=mv[:, 0:1], scalar=-1.0, in1=rstd,
            op0=MUL, op1=MUL)
        nc.scalar.activation(out=dst, in_=x_ap, func=IDENT,
                             bias=nmr, scale=rstd)
    if affine:
        nc.vector.tensor_mul(out=dst, in0=dst, in1=gb)
        nc.vector.tensor_add(out=out_ap, in0=dst, in1=beb)


# ---------------------------------------------------------------------------
# host side
# ---------------------------------------------------------------------------

_PROG_CACHE = {}


def get_program(S=2048, D=1024, F=4096, affine1=False, affine2=False):
    key = (S, D, F, affine1, affine2)
    if key not in _PROG_CACHE:
        _PROG_CACHE[key] = build_program(S, D, F, affine1, affine2)
    return _PROG_CACHE[key]


def make_in_maps(x, w1, b1, w2, b2, g1, be1, g2, be2, n_cores=8):
    import ml_dtypes
    FP8NP = ml_dtypes.float8_e4m3fn
    B, L, D = x.shape
    F = w1.shape[0]
    S = L
    H = D // E
    Lq = L // 2
    ST, LT, DT, FT = S // P, Lq // P, D // P, F // P
    x = np.asarray(x, np.float32)
    # w1t[ft, p, dc*128+m] = w1[ft*128+m, dc*128+p]
    w1t = np.ascontiguousarray(
        w1.astype(np.float32).reshape(FT, P, DT, P).transpose(0, 3, 2, 1)
        .reshape(FT, P, D)).astype(ml_dtypes.bfloat16)
    # w2 transposed [dt, p, ft*128+m] = w2[dt*128+m, ft*128+p] * S2,
    # first NF8 f-tiles in fp8 (DoubleRow), the rest in bf16 (the *32
    # scale is exact in bf16 and lets one uniform unscale serve both)
    NF8 = 2 * NFP8_FT2 if FFN2_FP8 else 0
    w2m = (w2.astype(np.float32).reshape(DT, P, FT, P)
           .transpose(0, 3, 2, 1).reshape(DT, P, FT, P)) * S2
    common = dict(w1t=w1t, b1=b1, b2=b2, g1=g1, be1=be1, g2=g2, be2=be2)
    if NF8:
        common["w2t8"] = np.ascontiguousarray(
            w2m[:, :, :NF8]).astype(FP8NP).reshape(DT, P, NF8 * P)
    if NF8 < FT:
        common["w2t16"] = np.ascontiguousarray(
            w2m[:, :, NF8:]).astype(ml_dtypes.bfloat16).reshape(
                DT, P, (FT - NF8) * P)
    in_maps = []
    for c in range(n_cores):
        b, half = c // 2, c % 2
        lo = half * Lq
        xq = x[b, lo:lo + Lq]
        xo = x[b, Lq - lo:2 * Lq - lo]
        xbl = np.concatenate([xq, xo], axis=0)
        # xT[t, p, s] = xbl[s, t*128+p]; xT2 has the two head-halves of
        # each d-tile swapped so a head's scores matmuls can alternate PE
        # row-groups (same-head row tiling)
        xT = np.ascontiguousarray(
            xbl.reshape(S, DT, P).transpose(1, 2, 0)).astype(
                ml_dtypes.bfloat16)
        xT2 = np.ascontiguousarray(
            np.roll(xT.reshape(DT, 2, E, S), 1, axis=1).reshape(DT, P, S))
        # v[u, p, h, :] = [xbl[u*128+p, h*64:(h+1)*64] | 1.0]
        # same-half key tiles (u < ST/2, contain the diagonal) in bf16,
        # cross-half tiles in fp8.
        va = np.ones((ST, P, H, W), np.float32)
        va[:, :, :, 0:E] = xbl.reshape(ST, P, H, E)
        v16 = np.ascontiguousarray(va[:ST // 2]).astype(
            ml_dtypes.bfloat16).reshape(ST // 2, P, H * W)
        v8 = np.ascontiguousarray(va[ST // 2:]).astype(FP8NP).reshape(
            ST // 2, P, H * W)
        xq32 = np.ascontiguousarray(xbl[0:Lq].reshape(LT, P, D))
        in_maps.append(dict(xT=xT, xT2=xT2, v16=v16, v8=v8, xq32=xq32,
                            **common))
    return in_maps


def kernel(x, w1, b1, w2, b2, g1, be1, g2, be2):
    from concourse.bass_utils import run_bass_kernel_spmd

    x = np.asarray(x, dtype=np.float32)
    B, L, D = x.shape
    F = w1.shape[0]
    Lq = L // 2
    n_cores = 2 * B
    g1 = np.asarray(g1, np.float32)
    be1 = np.asarray(be1, np.float32)
    g2 = np.asarray(g2, np.float32)
    be2 = np.asarray(be2, np.float32)
    affine1 = not (np.all(g1 == 1.0) and np.all(be1 == 0.0))
    affine2 = not (np.all(g2 == 1.0) and np.all(be2 == 0.0))
    nc = get_program(L, D, F, affine1, affine2)
    in_maps = make_in_maps(x, np.asarray(w1, np.float32),
                           np.asarray(b1, np.float32),
                           np.asarray(w2, np.float32),
                           np.asarray(b2, np.float32),
                           g1, be1, g2, be2, n_cores)
    res = run_bass_kernel_spmd(nc, in_maps, core_ids=list(range(n_cores)))
    outp = np.empty((B, L, D), dtype=np.float32)
    for c in range(n_cores):
        b, half = c // 2, c % 2
        outp[b, half * Lq:(half + 1) * Lq] = res.results[c]["out"]
    return outp
